# revision 52
# baseline (speedup 1.0000x reference)
"""GAT layer kernel for Trainium2 (8 NeuronCores, SPMD data-parallel over B).

Reference computation (per (b,t) slice, N=512 nodes, D=F=128):
    h = x_bt @ W; es = h@a_src; ed = h@a_dst
    e[i,j] = leaky_relu(es[i] + ed[j], 0.2) masked by adj|I
    alpha = row-softmax(e); out = elu(alpha @ h)

Device dataflow (v2) per (b,t), z kept transposed zT[j,i] so softmax
row-sums and aggregation contract over j via PSUM-accumulated matmuls:
    xT   : PE transposes of x chunks (bf16)           [d, n]
    ev   : WSD3.T @ xT -> [es; 0; ed] rows            [3, n]
    ev3  : one DVE tensor_scalar -> [es; ones; ed]; the rank-2 outer-sum
           matmul reads overlapping slices rows1:3 (lhsT) / rows0:2 (rhs)
    eadd : PSUM prefilled with mask bias (0/-1e9) via identity matmul
           (bf16), then K=2 rank-2 matmul adds ed[j]+es[i]
    prelu (Act, PSUM->SBUF f32) ; exp (Act, f32->bf16) => z = exp(e)^T
    s    : 16 tiny matmuls z_c^T @ ones -> s[i] column form [128,4]
           (ap_size=1, ~free on PE; lands in [i-partition] layout so no
           transposes are needed for the reciprocal)
    y    : x^T z accumulated over j-chunks            [d, i]  (h-free
           aggregation: u^T = W^T (x^T z), so h is never materialized)
    uT   : W.T @ y                                    [f, i]
    u    : PE transposes back -> [i, f]; v = u*r (DVE), t = exp(v) (Act),
           out = max(v, min(t,1)-1) = elu(v) (Pool)

Sharding: B=16 batches over 8 cores (2 per core), T=12 inside.
"""

import numpy as np

B, N, T, D, F = 16, 512, 12, 128, 128
NCORES = 8
B_PER_CORE = B // NCORES
NCH = N // 128  # 4 chunks of 128 nodes


def _build_program(reps=1):
    import concourse.bacc as bacc
    import concourse.tile as tile
    from concourse import mybir

    import os
    F32 = mybir.dt.float32
    F32R = mybir.dt.float32r
    BF16 = mybir.dt.bfloat16
    AF = mybir.ActivationFunctionType
    ALU = mybir.AluOpType

    nc = bacc.Bacc()

    x_h = nc.declare_dram_parameter("x", [B_PER_CORE, N, T, D], BF16, isOutput=False)
    # packed bf16 consts: [W | wsd | ident | ones] = 128+2+128+1 cols
    cb_h = nc.declare_dram_parameter("cb", [128, F + 2 + 128 + 1], BF16, isOutput=False)
    maskt_h = nc.declare_dram_parameter("maskt", [NCH, 128, N], BF16, isOutput=False)
    # f32 consts for the rank-1 broadcasts: [R1 (2xN) | L1 (2x128)]
    cf_h = nc.declare_dram_parameter("cf", [2, N + 128], F32R, isOutput=False)
    out_h = nc.declare_dram_parameter("out", [B_PER_CORE, N, T, F], F32, isOutput=True)

    NBT = B_PER_CORE * T

    with tile.TileContext(nc) as tc:
        with (
            tc.tile_pool(name="consts", bufs=1) as consts,
            tc.tile_pool(name="xbuf", bufs=1) as xbuf,
            tc.tile_pool(name="work", bufs=int(os.environ.get("K_WORK", "4"))) as work,
            tc.tile_pool(name="zf32", bufs=int(os.environ.get("K_ZF", "2"))) as zf32,
            tc.tile_pool(name="big", bufs=int(os.environ.get("K_BIG", "5"))) as big,
            tc.tile_pool(name="ps", bufs=1, space="PSUM") as ps,
        ):
            cb_sb = consts.tile([128, F + 2 + 128 + 1], BF16)
            mask_sb = consts.tile([128, NCH, N], BF16)
            cf_sb = consts.tile([2, N + 128], F32R)
            w_sb = cb_sb[:, 0:F]
            wsd_sb = cb_sb[:, F:F + 2]
            idb_sb = cb_sb[:, F + 2:F + 2 + 128]
            ones_sb = cb_sb[:, F + 2 + 128:F + 2 + 129]
            r1_sb = cf_sb[:, 0:N]        # [zeros; ones]
            l1_sb = cf_sb[:, N:N + 128]  # [ones; zeros]
            nc.sync.dma_start(out=cb_sb, in_=cb_h[:, :])
            nc.sync.dma_start(out=cf_sb, in_=cf_h[:, :])

            # ---- preload ALL of x (bf16); t=0 of b0 goes first (ahead of
            #      the 0.5MB mask) so compute starts early ----
            x_all = []
            for b in range(B_PER_CORE):
                xt = xbuf.tile([128, NCH, T, D], BF16, tag=f"x{b}")
                x_all.append(xt)
            xsrc = [x_h[b].rearrange("(c p) t d -> p c t d", p=128)
                    for b in range(B_PER_CORE)]
            nc.sync.dma_start(out=x_all[0][:, :, 0:1, :],
                              in_=xsrc[0][:, :, 0:1, :])
            for c in range(NCH):
                nc.sync.dma_start(out=mask_sb[:, c, :], in_=maskt_h[c, :, :])
            TQ = T // 4
            for b in range(B_PER_CORE):
                for th in range(4):
                    lo = max(th * TQ, 1 if b == 0 else 0)
                    nc.sync.dma_start(
                        out=x_all[b][:, :, lo:(th + 1) * TQ, :],
                        in_=xsrc[b][:, :, lo:(th + 1) * TQ, :])

            # per-bt state carried between pipeline stages
            st = [dict() for _ in range(NBT)]

            import os as _os
            B_XT = int(_os.environ.get("P_XT", "1"))
            B_MMB = int(_os.environ.get("P_MMB", "2"))

            def stage1(k):
                b, t = divmod(k, T)
                # transpose x -> xT [d, n] (bf16 in/out; bf16 PSUM gets the
                # DVE 2x_1p copy mode. gpsimd cannot touch PSUM.)
                xT_ps = ps.tile([128, NCH, 128], BF16, tag="tp", bufs=B_XT)
                for c in range(NCH):
                    nc.tensor.transpose(xT_ps[:, c, :], x_all[b][:, c, t, :], idb_sb)
                xT_sb = work.tile([128, NCH, 128], BF16, tag="xT_sb")
                nc.vector.tensor_copy(xT_sb, xT_ps)

                # ev = [es; ed] rows; ONE plain copy to SBUF. The outer sum
                # is then two rank-1 matmuls against const [ones;zeros] /
                # [zeros;ones] tiles (PE has slack, DVE is the wall).
                ev_ps = ps.tile([2, N], F32, tag="mmB", bufs=B_MMB)
                nc.tensor.matmul(
                    ev_ps, wsd_sb, xT_sb.rearrange("p a b -> p (a b)"),
                    start=True, stop=True)
                tev = work.tile([2, N], F32R, tag="tev")
                nc.vector.tensor_copy(tev, ev_ps)

                # eadd: mask prefill (bf16 identity matmul) + rank-2
                # outer-sum per chunk (f32 PSUM). Prelu split: chunks 0-2
                # one Act Prelu over 3 banks; chunk 3 via DVE 0.2-scale
                # (PSUM->SBUF f32) + Pool SBUF-only STT max(5t, t).
                z_pre = zf32.tile([128, NCH, N], BF16, tag="z_pre")

                def eadd(z_ps_c, c):
                    nc.tensor.matmul(
                        z_ps_c, idb_sb, mask_sb[:, c, :],
                        start=True, stop=False)
                    # += 1 (x) es  (broadcast es along rows)
                    nc.tensor.matmul(
                        z_ps_c, l1_sb, tev, start=False, stop=False)
                    # += ed (x) 1  (broadcast ed along cols)
                    nc.tensor.matmul(
                        z_ps_c, tev[:, c * 128:(c + 1) * 128], r1_sb,
                        start=False, stop=True)

                z_psA = ps.tile([128, 3, N], F32, tag="eaddA",
                                bufs=int(_os.environ.get("P_EA", "1")))
                for c in range(3):
                    eadd(z_psA[:, c, :], c)
                nc.scalar.activation(z_pre[:, 0:3, :], z_psA,
                                     AF.Prelu, alpha=0.2)
                z_psB = ps.tile([128, N], F32, tag="eaddB",
                                bufs=int(_os.environ.get("P_EB", "1")))
                eadd(z_psB, 3)
                # prelu(v) = t3 + 4*relu(t3) with t3 = 0.2v (Pool is
                # SBUF-only and has no scalar_tensor_tensor)
                t3_sb = work.tile([128, N], F32, tag="t3_sb")
                nc.vector.tensor_scalar(t3_sb, z_psB, 0.2, None, ALU.mult)
                r3_sb = work.tile([128, N], F32, tag="r3_sb")
                nc.gpsimd.tensor_scalar(r3_sb, t3_sb, 0.0, 4.0,
                                        ALU.max, ALU.mult)
                nc.gpsimd.tensor_tensor(
                    out=z_pre[:, 3, :], in0=t3_sb, in1=r3_sb, op=ALU.add)

                z_sb = big.tile([128, NCH, N], BF16, tag="z_sb")
                nc.scalar.activation(z_sb, z_pre, AF.Exp)
                st[k]["z_sb"] = z_sb

            def stage2(k):
                b, t = divmod(k, T)
                z_sb = st[k]["z_sb"]
                # y = x^T z accumulated over j chunks  [d, i] -- emitted
                # first so the Pool copy (and the uT chain behind it) can
                # start while PE grinds the 16 tiny s-matmuls
                y_ps = ps.tile([128, N], F32, tag="mmB", bufs=B_MMB)
                for cj in range(NCH):
                    nc.tensor.matmul(y_ps, x_all[b][:, cj, t, :],
                                     z_sb[:, cj, :],
                                     start=(cj == 0), stop=(cj == NCH - 1))
                y_sb = work.tile([128, N], BF16, tag="y_sb")
                nc.vector.tensor_copy(y_sb, y_ps)

                # s in [i-partition, chunk] column layout via tiny matmuls
                s_ps = ps.tile([128, NCH], F32, tag="mmB", bufs=B_MMB)
                for ci in range(NCH):
                    for cj in range(NCH):
                        nc.tensor.matmul(
                            s_ps[:, ci:ci + 1],
                            z_sb[:, cj, ci * 128:(ci + 1) * 128], ones_sb,
                            start=(cj == 0), stop=(cj == NCH - 1))
                r_cols = work.tile([128, NCH], F32, tag="r_cols")
                nc.vector.reciprocal_approx_fast(r_cols, s_ps)
                st[k]["y_sb"] = y_sb
                st[k]["r_cols"] = r_cols

            def stage3(k):
                b, t = divmod(k, T)
                y_sb, r_cols = st[k]["y_sb"], st[k]["r_cols"]
                # u chunks [i, f] directly: lhsT = y slice [d, i-chunk],
                # rhs = W [d, f]; bf16 so the ap=128 output has no penalty
                u_ps = ps.tile([128, NCH, F], F32, tag="u", bufs=1)
                for c in range(NCH):
                    nc.tensor.matmul(
                        u_ps[:, c, :], y_sb[:, c * 128:(c + 1) * 128], w_sb,
                        start=True, stop=True)
                v_sb = work.tile([128, NCH, F], BF16, tag="v_sb")
                for c in range(NCH):
                    nc.vector.tensor_scalar(
                        v_sb[:, c, :], u_ps[:, c, :],
                        r_cols[:, c:c + 1], None, ALU.mult)
                t_sb = work.tile([128, NCH, F], BF16, tag="t_sb")
                nc.scalar.activation(t_sb, v_sb, AF.Exp)
                e1_sb = work.tile([128, NCH, F], BF16, tag="e1_sb")
                nc.gpsimd.tensor_scalar(
                    e1_sb, t_sb, 1.0, -1.0, ALU.min, ALU.add)
                m_sb = work.tile([128, NCH, F], BF16, tag="m_sb")
                nc.vector.tensor_scalar(
                    m_sb, v_sb, 0.0, None, ALU.max)
                o_sb = work.tile([128, NCH, F], F32, tag="o_sb")
                nc.gpsimd.tensor_tensor(
                    out=o_sb, in0=m_sb, in1=e1_sb, op=ALU.add)
                o_dst = out_h[b, :, t, :].rearrange("(c p) f -> p c f", p=128)
                nc.sync.dma_start(out=o_dst, in_=o_sb)
                st[k].clear()

            # software-pipelined emission with stage lag
            LAG = int(os.environ.get("K_LAG", "3"))

            def body(_iv=None, unroll=1):
                for k in range(NBT + 2 * LAG):
                    if k >= 2 * LAG:
                        stage3(k - 2 * LAG)
                    if LAG <= k < NBT + LAG:
                        stage2(k - LAG)
                    if k < NBT:
                        stage1(k)

            if reps == 1:
                body()
            else:
                with tc.For_i(0, reps, 1) as _iv:
                    body(_iv)

    nc.finalize()
    return nc


def kernel(x, W, a_src, a_dst, adj):
    import ml_dtypes
    from concourse.bass_utils import run_bass_kernel_spmd

    BF = ml_dtypes.bfloat16
    x = np.ascontiguousarray(x, dtype=np.float32)
    W = np.ascontiguousarray(W, dtype=np.float32)
    a_src = np.asarray(a_src, dtype=np.float32)
    a_dst = np.asarray(a_dst, dtype=np.float32)
    adj = np.asarray(adj)

    mask = np.where((adj > 0) | np.eye(N, dtype=bool), 0.0, -1e9).astype(np.float32)  # [i, j]
    maskt = np.ascontiguousarray(mask.T.reshape(NCH, 128, N)).astype(BF)  # [jc, jl, i]
    wsd = np.stack([W @ a_src, W @ a_dst], axis=1)
    identb = np.eye(128, dtype=np.float32)
    onescol = np.ones((128, 1), np.float32)
    cb = np.concatenate([W, wsd, identb, onescol], axis=1).astype(BF)
    # cf = [R1 | L1]: R1 [2,N] = [zeros; ones], L1 [2,128] = [ones; zeros]
    cf = np.zeros((2, N + 128), np.float32)
    cf[1, 0:N] = 1.0
    cf[0, N:] = 1.0
    x16 = x.astype(BF)

    nc = _build_program()

    in_maps = []
    for c in range(NCORES):
        in_maps.append({
            "x": np.ascontiguousarray(x16[c * B_PER_CORE:(c + 1) * B_PER_CORE]),
            "cb": cb, "maskt": maskt, "cf": cf,
        })

    res = run_bass_kernel_spmd(nc, in_maps, list(range(NCORES)))
    out = np.concatenate([res.results[c]["out"] for c in range(NCORES)], axis=0)
    return out  # [B, N, T, F]


# revision 58
# speedup vs baseline: 1.3585x; 1.3585x over previous
"""GAT layer kernel for Trainium2 (8 NeuronCores, SPMD data-parallel over B).

Reference computation (per (b,t) slice, N=512 nodes, D=F=128):
    h = x_bt @ W; es = h@a_src; ed = h@a_dst
    e[i,j] = leaky_relu(es[i] + ed[j], 0.2) masked by adj|I
    alpha = row-softmax(e); out = elu(alpha @ h)

Device dataflow (v2) per (b,t), z kept transposed zT[j,i] so softmax
row-sums and aggregation contract over j via PSUM-accumulated matmuls:
    xT   : PE transposes of x chunks (bf16)           [d, n]
    ev   : WSD3.T @ xT -> [es; 0; ed] rows            [3, n]
    ev3  : one DVE tensor_scalar -> [es; ones; ed]; the rank-2 outer-sum
           matmul reads overlapping slices rows1:3 (lhsT) / rows0:2 (rhs)
    eadd : PSUM prefilled with mask bias (0/-1e9) via identity matmul
           (bf16), then K=2 rank-2 matmul adds ed[j]+es[i]
    prelu (Act, PSUM->SBUF f32) ; exp (Act, f32->bf16) => z = exp(e)^T
    s    : 16 tiny matmuls z_c^T @ ones -> s[i] column form [128,4]
           (ap_size=1, ~free on PE; lands in [i-partition] layout so no
           transposes are needed for the reciprocal)
    y    : x^T z accumulated over j-chunks            [d, i]  (h-free
           aggregation: u^T = W^T (x^T z), so h is never materialized)
    uT   : W.T @ y                                    [f, i]
    u    : PE transposes back -> [i, f]; v = u*r (DVE), t = exp(v) (Act),
           out = max(v, min(t,1)-1) = elu(v) (Pool)

Sharding: B=16 batches over 8 cores (2 per core), T=12 inside.
"""

import numpy as np

B, N, T, D, F = 16, 512, 12, 128, 128
NCORES = 8
B_PER_CORE = B // NCORES
NCH = N // 128  # 4 chunks of 128 nodes


def _build_program(reps=1):
    import concourse.bacc as bacc
    import concourse.tile as tile
    from concourse import mybir

    import os
    F32 = mybir.dt.float32
    F32R = mybir.dt.float32r
    BF16 = mybir.dt.bfloat16
    AF = mybir.ActivationFunctionType
    ALU = mybir.AluOpType

    nc = bacc.Bacc()

    x_h = nc.declare_dram_parameter("x", [B_PER_CORE, N, T, D], BF16, isOutput=False)
    # packed bf16 consts: [W | wsd(34) | ident | ones] = 128+34+128+1 cols
    cb_h = nc.declare_dram_parameter("cb", [128, F + 34 + 128 + 1], BF16, isOutput=False)
    maskt_h = nc.declare_dram_parameter("maskt", [NCH, 128, N], BF16, isOutput=False)
    sel_h = nc.declare_dram_parameter("sel", [34, 2], F32, isOutput=False)
    out_h = nc.declare_dram_parameter("out", [B_PER_CORE, N, T, F], F32, isOutput=True)

    NBT = B_PER_CORE * T

    with tile.TileContext(nc) as tc:
        with (
            tc.tile_pool(name="consts", bufs=1) as consts,
            tc.tile_pool(name="xbuf", bufs=1) as xbuf,
            tc.tile_pool(name="work", bufs=int(os.environ.get("K_WORK", "4"))) as work,
            tc.tile_pool(name="zf32", bufs=int(os.environ.get("K_ZF", "2"))) as zf32,
            tc.tile_pool(name="big", bufs=int(os.environ.get("K_BIG", "5"))) as big,
            tc.tile_pool(name="ps", bufs=1, space="PSUM") as ps,
        ):
            cb_sb = consts.tile([128, F + 34 + 128 + 1], BF16)
            mask_sb = consts.tile([128, NCH, N], BF16)
            sel_sb = consts.tile([34, 2], F32)
            w_sb = cb_sb[:, 0:F]
            wsd_sb = cb_sb[:, F:F + 34]
            idb_sb = cb_sb[:, F + 34:F + 34 + 128]
            ones_sb = cb_sb[:, F + 34 + 128:F + 34 + 129]
            nc.sync.dma_start(out=cb_sb, in_=cb_h[:, :])
            nc.sync.dma_start(out=sel_sb, in_=sel_h[:, :])

            # ---- preload ALL of x (bf16); t=0 of b0 goes first (ahead of
            #      the 0.5MB mask) so compute starts early ----
            x_all = []
            for b in range(B_PER_CORE):
                xt = xbuf.tile([128, NCH, T, D], BF16, tag=f"x{b}")
                x_all.append(xt)
            xsrc = [x_h[b].rearrange("(c p) t d -> p c t d", p=128)
                    for b in range(B_PER_CORE)]
            nc.sync.dma_start(out=x_all[0][:, :, 0:1, :],
                              in_=xsrc[0][:, :, 0:1, :])
            for c in range(NCH):
                nc.sync.dma_start(out=mask_sb[:, c, :], in_=maskt_h[c, :, :])
            TQ = T // 4
            for b in range(B_PER_CORE):
                for th in range(4):
                    lo = max(th * TQ, 1 if b == 0 else 0)
                    nc.sync.dma_start(
                        out=x_all[b][:, :, lo:(th + 1) * TQ, :],
                        in_=xsrc[b][:, :, lo:(th + 1) * TQ, :])

            # per-bt state carried between pipeline stages
            st = [dict() for _ in range(NBT)]

            import os as _os
            B_XT = int(_os.environ.get("P_XT", "1"))
            B_MMB = int(_os.environ.get("P_MMB", "2"))

            def stage1(k):
                b, t = divmod(k, T)
                # transpose x -> xT [d, n] (bf16 in/out; bf16 PSUM gets the
                # DVE 2x_1p copy mode. gpsimd cannot touch PSUM.)
                xT_ps = ps.tile([128, NCH, 128], BF16, tag="tp", bufs=B_XT)
                for c in range(NCH):
                    nc.tensor.transpose(xT_ps[:, c, :], x_all[b][:, c, t, :], idb_sb)
                xT_sb = work.tile([128, NCH, 128], BF16, tag="xT_sb")
                nc.vector.tensor_copy(xT_sb, xT_ps)

                # ev rows: [es; 0; ...] at 0:2 and [0; ed] at 32:34 (input
                # slice bases must be 0/32/64); two sel tensor_scalars give
                # ev_rhs=[es;ones], ev_lhs=[ones;ed], both base-0 tiles as
                # the rank-2 matmul requires equal base partitions
                ev_ps = ps.tile([34, N], F32, tag="mmB", bufs=B_MMB)
                nc.tensor.matmul(
                    ev_ps, wsd_sb, xT_sb.rearrange("p a b -> p (a b)"),
                    start=True, stop=True)
                ev_rhs = work.tile([2, N], F32R, tag="ev_rhs")
                ev_lhs = work.tile([2, N], F32R, tag="ev_lhs")
                nc.vector.tensor_scalar(
                    ev_rhs, ev_ps[0:2, :], sel_sb[0:2, 0:1], sel_sb[0:2, 1:2],
                    ALU.mult, ALU.add)
                nc.vector.tensor_scalar(
                    ev_lhs, ev_ps[32:34, :], sel_sb[32:34, 0:1],
                    sel_sb[32:34, 1:2], ALU.mult, ALU.add)

                # eadd: mask prefill (bf16 identity matmul) + rank-2
                # outer-sum per chunk (f32 PSUM). Prelu split: chunks 0-2
                # one Act Prelu over 3 banks; chunk 3 via DVE 0.2-scale
                # (PSUM->SBUF f32) + Pool SBUF-only STT max(5t, t).
                z_pre = zf32.tile([128, NCH, N], BF16, tag="z_pre")

                def eadd(z_ps_c, c):
                    nc.tensor.matmul(
                        z_ps_c, idb_sb, mask_sb[:, c, :],
                        start=True, stop=False)
                    nc.tensor.matmul(
                        z_ps_c, ev_lhs[:, c * 128:(c + 1) * 128],
                        ev_rhs, start=False, stop=True)

                z_psA = ps.tile([128, 3, N], F32, tag="eaddA",
                                bufs=int(_os.environ.get("P_EA", "1")))
                for c in range(3):
                    eadd(z_psA[:, c, :], c)
                nc.scalar.activation(z_pre[:, 0:3, :], z_psA,
                                     AF.Prelu, alpha=0.2)
                z_psB = ps.tile([128, N], F32, tag="eaddB",
                                bufs=int(_os.environ.get("P_EB", "1")))
                eadd(z_psB, 3)
                # prelu(v) = t3 + 4*relu(t3) with t3 = 0.2v (Pool is
                # SBUF-only and has no scalar_tensor_tensor)
                t3_sb = work.tile([128, N], F32, tag="t3_sb")
                nc.vector.tensor_scalar(t3_sb, z_psB, 0.2, None, ALU.mult)
                r3_sb = work.tile([128, N], F32, tag="r3_sb")
                nc.gpsimd.tensor_scalar(r3_sb, t3_sb, 0.0, 4.0,
                                        ALU.max, ALU.mult)
                nc.gpsimd.tensor_tensor(
                    out=z_pre[:, 3, :], in0=t3_sb, in1=r3_sb, op=ALU.add)

                z_sb = big.tile([128, NCH, N], BF16, tag="z_sb")
                nc.scalar.activation(z_sb, z_pre, AF.Exp)
                st[k]["z_sb"] = z_sb

            def stage2(k):
                b, t = divmod(k, T)
                z_sb = st[k]["z_sb"]
                # y = x^T z accumulated over j chunks  [d, i] -- emitted
                # first so the Pool copy (and the uT chain behind it) can
                # start while PE grinds the 16 tiny s-matmuls
                y_ps = ps.tile([128, N], F32, tag="mmB", bufs=B_MMB)
                for cj in range(NCH):
                    nc.tensor.matmul(y_ps, x_all[b][:, cj, t, :],
                                     z_sb[:, cj, :],
                                     start=(cj == 0), stop=(cj == NCH - 1))
                y_sb = work.tile([128, N], BF16, tag="y_sb")
                nc.vector.tensor_copy(y_sb, y_ps)

                # s in [i-partition, chunk] column layout via tiny matmuls
                s_ps = ps.tile([128, NCH], F32, tag="mmB", bufs=B_MMB)
                for ci in range(NCH):
                    for cj in range(NCH):
                        nc.tensor.matmul(
                            s_ps[:, ci:ci + 1],
                            z_sb[:, cj, ci * 128:(ci + 1) * 128], ones_sb,
                            start=(cj == 0), stop=(cj == NCH - 1))
                r_cols = work.tile([128, NCH], F32, tag="r_cols")
                nc.vector.reciprocal_approx_fast(r_cols, s_ps)
                st[k]["y_sb"] = y_sb
                st[k]["r_cols"] = r_cols

            def stage3(k):
                b, t = divmod(k, T)
                y_sb, r_cols = st[k]["y_sb"], st[k]["r_cols"]
                # u chunks [i, f] directly: lhsT = y slice [d, i-chunk],
                # rhs = W [d, f]; bf16 so the ap=128 output has no penalty
                u_ps = ps.tile([128, NCH, F], F32, tag="u", bufs=1)
                for c in range(NCH):
                    nc.tensor.matmul(
                        u_ps[:, c, :], y_sb[:, c * 128:(c + 1) * 128], w_sb,
                        start=True, stop=True)
                v_sb = work.tile([128, NCH, F], BF16, tag="v_sb")
                for c in range(NCH):
                    nc.vector.tensor_scalar(
                        v_sb[:, c, :], u_ps[:, c, :],
                        r_cols[:, c:c + 1], None, ALU.mult)
                t_sb = work.tile([128, NCH, F], BF16, tag="t_sb")
                nc.scalar.activation(t_sb, v_sb, AF.Exp)
                e1_sb = work.tile([128, NCH, F], BF16, tag="e1_sb")
                nc.gpsimd.tensor_scalar(
                    e1_sb, t_sb, 1.0, -1.0, ALU.min, ALU.add)
                m_sb = work.tile([128, NCH, F], BF16, tag="m_sb")
                nc.vector.tensor_scalar(
                    m_sb, v_sb, 0.0, None, ALU.max)
                o_sb = work.tile([128, NCH, F], F32, tag="o_sb")
                nc.gpsimd.tensor_tensor(
                    out=o_sb, in0=m_sb, in1=e1_sb, op=ALU.add)
                o_dst = out_h[b, :, t, :].rearrange("(c p) f -> p c f", p=128)
                nc.sync.dma_start(out=o_dst, in_=o_sb)
                st[k].clear()

            # software-pipelined emission with stage lag
            LAG = int(os.environ.get("K_LAG", "3"))

            def body(_iv=None, unroll=1):
                for k in range(NBT + 2 * LAG):
                    if k >= 2 * LAG:
                        stage3(k - 2 * LAG)
                    if LAG <= k < NBT + LAG:
                        stage2(k - LAG)
                    if k < NBT:
                        stage1(k)

            if reps == 1:
                body()
            else:
                with tc.For_i(0, reps, 1) as _iv:
                    body(_iv)

    nc.finalize()
    return nc


def kernel(x, W, a_src, a_dst, adj):
    import ml_dtypes
    from concourse.bass_utils import run_bass_kernel_spmd

    BF = ml_dtypes.bfloat16
    x = np.ascontiguousarray(x, dtype=np.float32)
    W = np.ascontiguousarray(W, dtype=np.float32)
    a_src = np.asarray(a_src, dtype=np.float32)
    a_dst = np.asarray(a_dst, dtype=np.float32)
    adj = np.asarray(adj)

    mask = np.where((adj > 0) | np.eye(N, dtype=bool), 0.0, -1e9).astype(np.float32)  # [i, j]
    maskt = np.ascontiguousarray(mask.T.reshape(NCH, 128, N)).astype(BF)  # [jc, jl, i]
    wsd = np.zeros((D, 34), np.float32)
    wsd[:, 0] = W @ a_src
    wsd[:, 33] = W @ a_dst
    identb = np.eye(128, dtype=np.float32)
    onescol = np.ones((128, 1), np.float32)
    cb = np.concatenate([W, wsd, identb, onescol], axis=1).astype(BF)
    # ev_rhs = ev_ps[0:2]*sel[0:2,0]+sel[0:2,1] = [es; ones]
    # ev_lhs = ev_ps[32:34]*sel[32:34,0]+sel[32:34,1] = [ones; ed]
    sel = np.zeros((34, 2), np.float32)
    sel[0] = [1.0, 0.0]
    sel[1] = [0.0, 1.0]
    sel[32] = [0.0, 1.0]
    sel[33] = [1.0, 0.0]
    x16 = x.astype(BF)

    nc = _build_program()

    in_maps = []
    for c in range(NCORES):
        in_maps.append({
            "x": np.ascontiguousarray(x16[c * B_PER_CORE:(c + 1) * B_PER_CORE]),
            "cb": cb, "maskt": maskt, "sel": sel,
        })

    res = run_bass_kernel_spmd(nc, in_maps, list(range(NCORES)))
    out = np.concatenate([res.results[c]["out"] for c in range(NCORES)], axis=0)
    return out  # [B, N, T, F]


# revision 69
# speedup vs baseline: 1.3837x; 1.0186x over previous
"""GAT layer kernel for Trainium2 (8 NeuronCores, SPMD data-parallel over B).

Reference computation (per (b,t) slice, N=512 nodes, D=F=128):
    h = x_bt @ W; es = h@a_src; ed = h@a_dst
    e[i,j] = leaky_relu(es[i] + ed[j], 0.2) masked by adj|I
    alpha = row-softmax(e); out = elu(alpha @ h)

Device dataflow per (b,t), z kept transposed zT[j,i] so softmax row-sums
and aggregation contract over j via PSUM-accumulated matmuls. Work is
spread across all four engines (Act/DVE are the walls; gpsimd=Pool is
SBUF-only; DVE gets 2x_1p on bf16 PSUM reads):
    xT    : PE transposes of x chunks (bf16 PSUM) -> DVE 2x copy  [d, n]
    ev    : wsd.T @ xT -> rows es@0, ed@32 of [34, n] f32 PSUM; two DVE
            sel tensor_scalars -> ev_rhs=[es;ones], ev_lhs=[ones;ed]
            (partition-slice bases must be 0/32/64, operands equal-base)
    eadd  : per chunk: PSUM <- mask bias (0/-1e9, bf16 identity matmul)
            then K=2 rank-2 matmul adds ed[j]+es[i].
    prelu : chunks 0-2 one Act Prelu (3 PSUM banks); chunk 3 via DVE
            t3=0.2v then Pool r3=4*relu(t3), z3=t3+r3 (Pool has no PSUM
            access and only add/mult tensor_tensor)
    exp   : Act, 2 halves (lets stage2's PE work start early) -> z bf16
    s     : 16 tiny ap=1 matmuls z_c^T @ ones -> s columns [128,4]
            (engine-free on PE; no transposes needed for the reciprocal)
    y     : x^T z accumulated over j-chunks  [d, i]  (h-free aggregation
            u = (x^T z)^T-chunks @ W, so h/uT are never materialized)
    u     : per i-chunk matmul y_slice^T @ W (bf16, no ap<256 penalty)
    elu   : v=r*u (DVE), t=exp(v) (Act), e1=min(t,1)-1 (Pool),
            m=relu(v) (DVE 4x), out = m+e1 (Pool add) -> f32 DMA

Sharding: B=16 batches over 8 cores (2 per core), T=12 inside.
Timing (TimelineSim): 126.6us vs 166.8us baseline; HW rel err 8.8e-3.
"""

import numpy as np

B, N, T, D, F = 16, 512, 12, 128, 128
NCORES = 8
B_PER_CORE = B // NCORES
NCH = N // 128  # 4 chunks of 128 nodes


def _build_program(reps=1):
    import concourse.bacc as bacc
    import concourse.tile as tile
    from concourse import mybir

    import os
    F32 = mybir.dt.float32
    F32R = mybir.dt.float32r
    BF16 = mybir.dt.bfloat16
    AF = mybir.ActivationFunctionType
    ALU = mybir.AluOpType

    nc = bacc.Bacc()

    x_h = nc.declare_dram_parameter("x", [B_PER_CORE, N, T, D], BF16, isOutput=False)
    # packed bf16 consts: [W | wsd(34) | ident | ones] = 128+34+128+1 cols
    cb_h = nc.declare_dram_parameter("cb", [128, F + 34 + 128 + 1], BF16, isOutput=False)
    maskt_h = nc.declare_dram_parameter("maskt", [NCH, 128, N], BF16, isOutput=False)
    sel_h = nc.declare_dram_parameter("sel", [34, 2], F32, isOutput=False)
    out_h = nc.declare_dram_parameter("out", [B_PER_CORE, N, T, F], F32, isOutput=True)

    NBT = B_PER_CORE * T

    with tile.TileContext(nc) as tc:
        with (
            tc.tile_pool(name="consts", bufs=1) as consts,
            tc.tile_pool(name="xbuf", bufs=1) as xbuf,
            tc.tile_pool(name="work", bufs=int(os.environ.get("K_WORK", "6"))) as work,
            tc.tile_pool(name="zf32", bufs=int(os.environ.get("K_ZF", "2"))) as zf32,
            tc.tile_pool(name="big", bufs=int(os.environ.get("K_BIG", "6"))) as big,
            tc.tile_pool(name="ps", bufs=1, space="PSUM") as ps,
        ):
            cb_sb = consts.tile([128, F + 34 + 128 + 1], BF16)
            mask_sb = consts.tile([128, NCH, N], BF16)
            sel_sb = consts.tile([34, 2], F32)
            w_sb = cb_sb[:, 0:F]
            wsd_sb = cb_sb[:, F:F + 34]
            idb_sb = cb_sb[:, F + 34:F + 34 + 128]
            ones_sb = cb_sb[:, F + 34 + 128:F + 34 + 129]
            nc.sync.dma_start(out=cb_sb, in_=cb_h[:, :])
            nc.sync.dma_start(out=sel_sb, in_=sel_h[:, :])

            # ---- preload ALL of x (bf16); t=0 of b0 goes first (ahead of
            #      the 0.5MB mask) so compute starts early ----
            x_all = []
            for b in range(B_PER_CORE):
                xt = xbuf.tile([128, NCH, T, D], BF16, tag=f"x{b}")
                x_all.append(xt)
            xsrc = [x_h[b].rearrange("(c p) t d -> p c t d", p=128)
                    for b in range(B_PER_CORE)]
            nc.sync.dma_start(out=x_all[0][:, :, 0:1, :],
                              in_=xsrc[0][:, :, 0:1, :])
            for c in range(NCH):
                nc.sync.dma_start(out=mask_sb[:, c, :], in_=maskt_h[c, :, :])
            TQ = T // 4
            for b in range(B_PER_CORE):
                for th in range(4):
                    lo = max(th * TQ, 1 if b == 0 else 0)
                    nc.sync.dma_start(
                        out=x_all[b][:, :, lo:(th + 1) * TQ, :],
                        in_=xsrc[b][:, :, lo:(th + 1) * TQ, :])

            # per-bt state carried between pipeline stages
            st = [dict() for _ in range(NBT)]

            import os as _os
            B_XT = int(_os.environ.get("P_XT", "1"))
            B_MMB = int(_os.environ.get("P_MMB", "2"))

            def stage1(k):
                b, t = divmod(k, T)
                # transpose x -> xT [d, n] (bf16 in/out; bf16 PSUM gets the
                # DVE 2x_1p copy mode. gpsimd cannot touch PSUM.)
                xT_ps = ps.tile([128, NCH, 128], BF16, tag="tp", bufs=B_XT)
                for c in range(NCH):
                    nc.tensor.transpose(xT_ps[:, c, :], x_all[b][:, c, t, :], idb_sb)
                xT_sb = work.tile([128, NCH, 128], BF16, tag="xT_sb")
                nc.vector.tensor_copy(xT_sb, xT_ps)

                # ev rows: [es; 0; ...] at 0:2 and [0; ed] at 32:34 (input
                # slice bases must be 0/32/64); two sel tensor_scalars give
                # ev_rhs=[es;ones], ev_lhs=[ones;ed], both base-0 tiles as
                # the rank-2 matmul requires equal base partitions
                ev_ps = ps.tile([34, N], F32, tag="mmB", bufs=B_MMB)
                nc.tensor.matmul(
                    ev_ps, wsd_sb, xT_sb.rearrange("p a b -> p (a b)"),
                    start=True, stop=True)
                ev_rhs = work.tile([2, N], F32R, tag="ev_rhs")
                ev_lhs = work.tile([2, N], F32R, tag="ev_lhs")
                nc.vector.tensor_scalar(
                    ev_rhs, ev_ps[0:2, :], sel_sb[0:2, 0:1], sel_sb[0:2, 1:2],
                    ALU.mult, ALU.add)
                nc.vector.tensor_scalar(
                    ev_lhs, ev_ps[32:34, :], sel_sb[32:34, 0:1],
                    sel_sb[32:34, 1:2], ALU.mult, ALU.add)

                # eadd: mask prefill (bf16 identity matmul) + rank-2
                # outer-sum per chunk (f32 PSUM). Prelu split: chunks 0-2
                # one Act Prelu over 3 banks; chunk 3 via DVE 0.2-scale
                # (PSUM->SBUF f32) + Pool SBUF-only STT max(5t, t).
                z_pre = zf32.tile([128, NCH, N], BF16, tag="z_pre")

                def eadd(z_ps_c, c):
                    nc.tensor.matmul(
                        z_ps_c, idb_sb, mask_sb[:, c, :],
                        start=True, stop=False)
                    nc.tensor.matmul(
                        z_ps_c, ev_lhs[:, c * 128:(c + 1) * 128],
                        ev_rhs, start=False, stop=True)

                z_psA = ps.tile([128, 3, N], F32, tag="eaddA",
                                bufs=int(_os.environ.get("P_EA", "1")))
                for c in range(3):
                    eadd(z_psA[:, c, :], c)
                nc.scalar.activation(z_pre[:, 0:3, :], z_psA,
                                     AF.Prelu, alpha=0.2)
                z_psB = ps.tile([128, N], F32, tag="eaddB",
                                bufs=int(_os.environ.get("P_EB", "1")))
                eadd(z_psB, 3)
                # prelu(v) = t3 + 4*relu(t3) with t3 = 0.2v (Pool is
                # SBUF-only and has no scalar_tensor_tensor)
                t3_sb = work.tile([128, N], F32, tag="t3_sb")
                nc.vector.tensor_scalar(t3_sb, z_psB, 0.2, None, ALU.mult)
                r3_sb = work.tile([128, N], F32, tag="r3_sb")
                nc.gpsimd.tensor_scalar(r3_sb, t3_sb, 0.0, 4.0,
                                        ALU.max, ALU.mult)
                nc.gpsimd.tensor_tensor(
                    out=z_pre[:, 3, :], in0=t3_sb, in1=r3_sb, op=ALU.add)

                z_sb = big.tile([128, NCH, N], BF16, tag="z_sb")
                nexp = int(_os.environ.get("K_EXPH", "2"))
                if nexp == 1:
                    nc.scalar.activation(z_sb, z_pre, AF.Exp)
                else:
                    step = NCH // nexp
                    for h in range(nexp):
                        lo, hi = h * step, (h + 1) * step
                        nc.scalar.activation(z_sb[:, lo:hi, :],
                                             z_pre[:, lo:hi, :], AF.Exp)
                st[k]["z_sb"] = z_sb

            def stage2(k):
                b, t = divmod(k, T)
                z_sb = st[k]["z_sb"]
                # y = x^T z accumulated over j chunks  [d, i] -- emitted
                # first so the Pool copy (and the uT chain behind it) can
                # start while PE grinds the 16 tiny s-matmuls
                y_ps = ps.tile([128, N], F32, tag="mmB", bufs=B_MMB)
                for cj in range(NCH):
                    nc.tensor.matmul(y_ps, x_all[b][:, cj, t, :],
                                     z_sb[:, cj, :],
                                     start=(cj == 0), stop=(cj == NCH - 1))
                y_sb = work.tile([128, N], BF16, tag="y_sb")
                nc.vector.tensor_copy(y_sb, y_ps)

                # s in [i-partition, chunk] column layout via tiny matmuls
                s_ps = ps.tile([128, NCH], F32, tag="mmB", bufs=B_MMB)
                for ci in range(NCH):
                    for cj in range(NCH):
                        nc.tensor.matmul(
                            s_ps[:, ci:ci + 1],
                            z_sb[:, cj, ci * 128:(ci + 1) * 128], ones_sb,
                            start=(cj == 0), stop=(cj == NCH - 1))
                r_cols = work.tile([128, NCH], F32, tag="r_cols")
                nc.vector.reciprocal_approx_fast(r_cols, s_ps)
                st[k]["y_sb"] = y_sb
                st[k]["r_cols"] = r_cols

            def stage3(k):
                b, t = divmod(k, T)
                y_sb, r_cols = st[k]["y_sb"], st[k]["r_cols"]
                # u chunks [i, f] directly: lhsT = y slice [d, i-chunk],
                # rhs = W [d, f]; bf16 so the ap=128 output has no penalty
                u_ps = ps.tile([128, NCH, F], F32, tag="u", bufs=1)
                for c in range(NCH):
                    nc.tensor.matmul(
                        u_ps[:, c, :], y_sb[:, c * 128:(c + 1) * 128], w_sb,
                        start=True, stop=True)
                v_sb = work.tile([128, NCH, F], BF16, tag="v_sb")
                for c in range(NCH):
                    nc.vector.tensor_scalar(
                        v_sb[:, c, :], u_ps[:, c, :],
                        r_cols[:, c:c + 1], None, ALU.mult)
                t_sb = work.tile([128, NCH, F], BF16, tag="t_sb")
                nc.scalar.activation(t_sb, v_sb, AF.Exp)
                e1_sb = work.tile([128, NCH, F], BF16, tag="e1_sb")
                nc.gpsimd.tensor_scalar(
                    e1_sb, t_sb, 1.0, -1.0, ALU.min, ALU.add)
                m_sb = work.tile([128, NCH, F], BF16, tag="m_sb")
                nc.vector.tensor_scalar(
                    m_sb, v_sb, 0.0, None, ALU.max)
                o_sb = work.tile([128, NCH, F], F32, tag="o_sb")
                nc.gpsimd.tensor_tensor(
                    out=o_sb, in0=m_sb, in1=e1_sb, op=ALU.add)
                o_dst = out_h[b, :, t, :].rearrange("(c p) f -> p c f", p=128)
                nc.sync.dma_start(out=o_dst, in_=o_sb)
                st[k].clear()

            # software-pipelined emission with stage lag
            LAG = int(os.environ.get("K_LAG", "3"))

            def body(_iv=None, unroll=1):
                for k in range(NBT + 2 * LAG):
                    if k >= 2 * LAG:
                        stage3(k - 2 * LAG)
                    if LAG <= k < NBT + LAG:
                        stage2(k - LAG)
                    if k < NBT:
                        stage1(k)

            if reps == 1:
                body()
            else:
                with tc.For_i(0, reps, 1) as _iv:
                    body(_iv)

    nc.finalize()
    return nc


def kernel(x, W, a_src, a_dst, adj):
    import ml_dtypes
    from concourse.bass_utils import run_bass_kernel_spmd

    BF = ml_dtypes.bfloat16
    x = np.ascontiguousarray(x, dtype=np.float32)
    W = np.ascontiguousarray(W, dtype=np.float32)
    a_src = np.asarray(a_src, dtype=np.float32)
    a_dst = np.asarray(a_dst, dtype=np.float32)
    adj = np.asarray(adj)

    mask = np.where((adj > 0) | np.eye(N, dtype=bool), 0.0, -1e9).astype(np.float32)  # [i, j]
    maskt = np.ascontiguousarray(mask.T.reshape(NCH, 128, N)).astype(BF)  # [jc, jl, i]
    wsd = np.zeros((D, 34), np.float32)
    wsd[:, 0] = W @ a_src
    wsd[:, 33] = W @ a_dst
    identb = np.eye(128, dtype=np.float32)
    onescol = np.ones((128, 1), np.float32)
    cb = np.concatenate([W, wsd, identb, onescol], axis=1).astype(BF)
    # ev_rhs = ev_ps[0:2]*sel[0:2,0]+sel[0:2,1] = [es; ones]
    # ev_lhs = ev_ps[32:34]*sel[32:34,0]+sel[32:34,1] = [ones; ed]
    sel = np.zeros((34, 2), np.float32)
    sel[0] = [1.0, 0.0]
    sel[1] = [0.0, 1.0]
    sel[32] = [0.0, 1.0]
    sel[33] = [1.0, 0.0]
    x16 = x.astype(BF)

    nc = _build_program()

    in_maps = []
    for c in range(NCORES):
        in_maps.append({
            "x": np.ascontiguousarray(x16[c * B_PER_CORE:(c + 1) * B_PER_CORE]),
            "cb": cb, "maskt": maskt, "sel": sel,
        })

    res = run_bass_kernel_spmd(nc, in_maps, list(range(NCORES)))
    out = np.concatenate([res.results[c]["out"] for c in range(NCORES)], axis=0)
    return out  # [B, N, T, F]


# revision 76
# speedup vs baseline: 1.4049x; 1.0153x over previous
"""GAT layer kernel for Trainium2 (8 NeuronCores, SPMD data-parallel over B).

Reference computation (per (b,t) slice, N=512 nodes, D=F=128):
    h = x_bt @ W; es = h@a_src; ed = h@a_dst
    e[i,j] = leaky_relu(es[i] + ed[j], 0.2) masked by adj|I
    alpha = row-softmax(e); out = elu(alpha @ h)

Device dataflow per (b,t), z kept transposed zT[j,i] so softmax row-sums
and aggregation contract over j via PSUM-accumulated matmuls. Work is
spread across all four engines (Act/DVE are the walls; gpsimd=Pool is
SBUF-only; DVE gets 2x_1p on bf16 PSUM reads):
    xT    : PE transposes of x chunks (bf16 PSUM) -> DVE 2x copy  [d, n]
    ev    : wsd.T @ xT -> rows es@0, ed@32 of [34, n] f32 PSUM; two DVE
            sel tensor_scalars -> ev_rhs=[es;ones], ev_lhs=[ones;ed]
            (partition-slice bases must be 0/32/64, operands equal-base)
    eadd  : per chunk: PSUM <- mask bias (0/-1e9, bf16 identity matmul)
            then K=2 rank-2 matmul adds ed[j]+es[i].
    prelu : chunks 0-2 one Act Prelu (3 PSUM banks); chunk 3 via DVE
            t3=0.2v then Pool r3=4*relu(t3), z3=t3+r3 (Pool has no PSUM
            access and only add/mult tensor_tensor)
    exp   : Act, 2 halves (lets stage2's PE work start early) -> z bf16
    s     : 16 tiny ap=1 matmuls z_c^T @ ones -> s columns [128,4]
            (engine-free on PE; no transposes needed for the reciprocal)
    y     : x^T z accumulated over j-chunks  [d, i]  (h-free aggregation
            u = (x^T z)^T-chunks @ W, so h/uT are never materialized)
    u     : per i-chunk matmul y_slice^T @ W (bf16, no ap<256 penalty)
    elu   : v=r*u (DVE), t=exp(v) (Act), e1=min(t,1)-1 (Pool),
            m=relu(v) (DVE 4x), out = m+e1 (Pool add) -> f32 DMA

Sharding: B=16 batches over 8 cores (2 per core), T=12 inside.
Timing (TimelineSim): 124.7us vs 166.8us baseline; HW rel err 8.8e-3.
"""

import numpy as np

B, N, T, D, F = 16, 512, 12, 128, 128
NCORES = 8
B_PER_CORE = B // NCORES
NCH = N // 128  # 4 chunks of 128 nodes


def _build_program(reps=1):
    import concourse.bacc as bacc
    import concourse.tile as tile
    from concourse import mybir

    import os
    F32 = mybir.dt.float32
    F32R = mybir.dt.float32r
    BF16 = mybir.dt.bfloat16
    AF = mybir.ActivationFunctionType
    ALU = mybir.AluOpType

    nc = bacc.Bacc()

    x_h = nc.declare_dram_parameter("x", [B_PER_CORE, N, T, D], BF16, isOutput=False)
    # packed bf16 consts: [W | wsd(34) | ident | ones] = 128+34+128+1 cols
    cb_h = nc.declare_dram_parameter("cb", [128, F + 34 + 128 + 1], BF16, isOutput=False)
    maskt_h = nc.declare_dram_parameter("maskt", [NCH, 128, N], BF16, isOutput=False)
    sel_h = nc.declare_dram_parameter("sel", [34, 2], F32, isOutput=False)
    out_h = nc.declare_dram_parameter("out", [B_PER_CORE, N, T, F], F32, isOutput=True)

    NBT = B_PER_CORE * T

    with tile.TileContext(nc) as tc:
        with (
            tc.tile_pool(name="consts", bufs=1) as consts,
            tc.tile_pool(name="xbuf", bufs=1) as xbuf,
            tc.tile_pool(name="work", bufs=int(os.environ.get("K_WORK", "6"))) as work,
            tc.tile_pool(name="zf32", bufs=int(os.environ.get("K_ZF", "2"))) as zf32,
            tc.tile_pool(name="big", bufs=int(os.environ.get("K_BIG", "6"))) as big,
            tc.tile_pool(name="ps", bufs=1, space="PSUM") as ps,
        ):
            cb_sb = consts.tile([128, F + 34 + 128 + 1], BF16)
            mask_sb = consts.tile([128, NCH, N], BF16)
            sel_sb = consts.tile([34, 2], F32)
            w_sb = cb_sb[:, 0:F]
            wsd_sb = cb_sb[:, F:F + 34]
            idb_sb = cb_sb[:, F + 34:F + 34 + 128]
            ones_sb = cb_sb[:, F + 34 + 128:F + 34 + 129]
            nc.sync.dma_start(out=cb_sb, in_=cb_h[:, :])
            nc.sync.dma_start(out=sel_sb, in_=sel_h[:, :])

            # ---- preload ALL of x (bf16); t=0 of b0 goes first (ahead of
            #      the 0.5MB mask) so compute starts early ----
            x_all = []
            for b in range(B_PER_CORE):
                xt = xbuf.tile([128, NCH, T, D], BF16, tag=f"x{b}")
                x_all.append(xt)
            xsrc = [x_h[b].rearrange("(c p) t d -> p c t d", p=128)
                    for b in range(B_PER_CORE)]
            nc.sync.dma_start(out=x_all[0][:, :, 0:1, :],
                              in_=xsrc[0][:, :, 0:1, :])
            for c in range(NCH):
                nc.sync.dma_start(out=mask_sb[:, c, :], in_=maskt_h[c, :, :])
            TQ = T // 4
            for b in range(B_PER_CORE):
                for th in range(4):
                    lo = max(th * TQ, 1 if b == 0 else 0)
                    nc.sync.dma_start(
                        out=x_all[b][:, :, lo:(th + 1) * TQ, :],
                        in_=xsrc[b][:, :, lo:(th + 1) * TQ, :])

            # per-bt state carried between pipeline stages
            st = [dict() for _ in range(NBT)]

            import os as _os
            B_XT = int(_os.environ.get("P_XT", "1"))
            B_MMB = int(_os.environ.get("P_MMB", "2"))

            def stage1(k):
                b, t = divmod(k, T)
                # transpose x -> xT [d, n] (bf16 in/out; bf16 PSUM gets the
                # DVE 2x_1p copy mode. gpsimd cannot touch PSUM.)
                xT_ps = ps.tile([128, NCH, 128], BF16, tag="tp", bufs=B_XT)
                for c in range(NCH):
                    nc.tensor.transpose(xT_ps[:, c, :], x_all[b][:, c, t, :], idb_sb)
                xT_sb = work.tile([128, NCH, 128], BF16, tag="xT_sb")
                nc.vector.tensor_copy(xT_sb, xT_ps)

                # ev rows: [es; 0; ...] at 0:2 and [0; ed] at 32:34 (input
                # slice bases must be 0/32/64); two sel tensor_scalars give
                # ev_rhs=[es;ones], ev_lhs=[ones;ed], both base-0 tiles as
                # the rank-2 matmul requires equal base partitions
                ev_ps = ps.tile([34, N], F32, tag="mmB", bufs=B_MMB)
                nc.tensor.matmul(
                    ev_ps, wsd_sb, xT_sb.rearrange("p a b -> p (a b)"),
                    start=True, stop=True)
                ev_rhs = work.tile([2, N], F32R, tag="ev_rhs")
                ev_lhs = work.tile([2, N], F32R, tag="ev_lhs")
                nc.vector.tensor_scalar(
                    ev_rhs, ev_ps[0:2, :], sel_sb[0:2, 0:1], sel_sb[0:2, 1:2],
                    ALU.mult, ALU.add)
                nc.vector.tensor_scalar(
                    ev_lhs, ev_ps[32:34, :], sel_sb[32:34, 0:1],
                    sel_sb[32:34, 1:2], ALU.mult, ALU.add)

                # eadd: mask prefill (bf16 identity matmul) + rank-2
                # outer-sum per chunk (f32 PSUM). Prelu split: chunks 0-2
                # one Act Prelu over 3 banks; chunk 3 via DVE 0.2-scale
                # (PSUM->SBUF f32) + Pool SBUF-only STT max(5t, t).
                z_pre = zf32.tile([128, NCH, N], BF16, tag="z_pre")

                def eadd(z_ps_c, c):
                    nc.tensor.matmul(
                        z_ps_c, idb_sb, mask_sb[:, c, :],
                        start=True, stop=False)
                    nc.tensor.matmul(
                        z_ps_c, ev_lhs[:, c * 128:(c + 1) * 128],
                        ev_rhs, start=False, stop=True)

                z_psA = ps.tile([128, 3, N], F32, tag="eaddA",
                                bufs=int(_os.environ.get("P_EA", "1")))
                for c in range(3):
                    eadd(z_psA[:, c, :], c)
                nc.scalar.activation(z_pre[:, 0:3, :], z_psA,
                                     AF.Prelu, alpha=0.2)
                z_psB = ps.tile([128, N], F32, tag="eaddB",
                                bufs=int(_os.environ.get("P_EB", "1")))
                eadd(z_psB, 3)
                # prelu(v) = t3 + 4*relu(t3) with t3 = 0.2v (Pool is
                # SBUF-only and has no scalar_tensor_tensor)
                t3_sb = work.tile([128, N], F32, tag="t3_sb")
                nc.vector.tensor_scalar(t3_sb, z_psB, 0.2, None, ALU.mult)
                r3_sb = work.tile([128, N], F32, tag="r3_sb")
                nc.gpsimd.tensor_scalar(r3_sb, t3_sb, 0.0, 4.0,
                                        ALU.max, ALU.mult)
                nc.gpsimd.tensor_tensor(
                    out=z_pre[:, 3, :], in0=t3_sb, in1=r3_sb, op=ALU.add)

                z_sb = big.tile([128, NCH, N], BF16, tag="z_sb")
                nexp = int(_os.environ.get("K_EXPH", "2"))
                if nexp == 1:
                    nc.scalar.activation(z_sb, z_pre, AF.Exp)
                else:
                    step = NCH // nexp
                    for h in range(nexp):
                        lo, hi = h * step, (h + 1) * step
                        nc.scalar.activation(z_sb[:, lo:hi, :],
                                             z_pre[:, lo:hi, :], AF.Exp)
                st[k]["z_sb"] = z_sb

            def stage2(k):
                b, t = divmod(k, T)
                z_sb = st[k]["z_sb"]
                # y = x^T z accumulated over j chunks  [d, i] -- emitted
                # first so the Pool copy (and the uT chain behind it) can
                # start while PE grinds the 16 tiny s-matmuls
                y_ps = ps.tile([128, N], F32, tag="mmB", bufs=B_MMB)
                for cj in range(NCH):
                    nc.tensor.matmul(y_ps, x_all[b][:, cj, t, :],
                                     z_sb[:, cj, :],
                                     start=(cj == 0), stop=(cj == NCH - 1))
                y_sb = work.tile([128, N], BF16, tag="y_sb")
                nc.vector.tensor_copy(y_sb, y_ps)

                # s in [i-partition, chunk] column layout via tiny matmuls
                s_ps = ps.tile([128, NCH], F32, tag="mmB", bufs=B_MMB)
                for ci in range(NCH):
                    for cj in range(NCH):
                        nc.tensor.matmul(
                            s_ps[:, ci:ci + 1],
                            z_sb[:, cj, ci * 128:(ci + 1) * 128], ones_sb,
                            start=(cj == 0), stop=(cj == NCH - 1))
                r_cols = work.tile([128, NCH], F32, tag="r_cols")
                nc.vector.reciprocal_approx_fast(r_cols, s_ps)
                st[k]["y_sb"] = y_sb
                st[k]["r_cols"] = r_cols

            def stage3(k):
                b, t = divmod(k, T)
                y_sb, r_cols = st[k]["y_sb"], st[k]["r_cols"]
                # u chunks [i, f] directly: lhsT = y slice [d, i-chunk],
                # rhs = W [d, f]; bf16 so the ap=128 output has no penalty
                u_ps = ps.tile([128, NCH, F], F32, tag="u", bufs=1)
                for c in range(NCH):
                    nc.tensor.matmul(
                        u_ps[:, c, :], y_sb[:, c * 128:(c + 1) * 128], w_sb,
                        start=True, stop=True)
                v_sb = work.tile([128, NCH, F], BF16, tag="v_sb")
                for c in range(NCH):
                    nc.vector.tensor_scalar(
                        v_sb[:, c, :], u_ps[:, c, :],
                        r_cols[:, c:c + 1], None, ALU.mult)
                t_sb = work.tile([128, NCH, F], BF16, tag="t_sb")
                nc.scalar.activation(t_sb, v_sb, AF.Exp)
                e1_sb = work.tile([128, NCH, F], BF16, tag="e1_sb")
                nc.gpsimd.tensor_scalar(
                    e1_sb, t_sb, 1.0, -1.0, ALU.min, ALU.add)
                m_sb = work.tile([128, NCH, F], BF16, tag="m_sb")
                nc.vector.tensor_scalar(
                    m_sb, v_sb, 0.0, None, ALU.max)
                o_sb = work.tile([128, NCH, F], F32, tag="o_sb")
                nc.gpsimd.tensor_tensor(
                    out=o_sb, in0=m_sb, in1=e1_sb, op=ALU.add)
                o_dst = out_h[b, :, t, :].rearrange("(c p) f -> p c f", p=128)
                nc.sync.dma_start(out=o_dst, in_=o_sb)
                st[k].clear()

            # software-pipelined emission with stage lag; stage3 trails
            # stage2 by LAG3 iterations (shorter drain than 2*LAG)
            LAG = int(os.environ.get("K_LAG", "4"))
            LAG3 = LAG + int(os.environ.get("K_LAG3", "1"))

            def body(_iv=None, unroll=1):
                for k in range(NBT + LAG3):
                    if k >= LAG3:
                        stage3(k - LAG3)
                    if LAG <= k < NBT + LAG:
                        stage2(k - LAG)
                    if k < NBT:
                        stage1(k)

            if reps == 1:
                body()
            else:
                with tc.For_i(0, reps, 1) as _iv:
                    body(_iv)

    nc.finalize()
    return nc


def kernel(x, W, a_src, a_dst, adj):
    import ml_dtypes
    from concourse.bass_utils import run_bass_kernel_spmd

    BF = ml_dtypes.bfloat16
    x = np.ascontiguousarray(x, dtype=np.float32)
    W = np.ascontiguousarray(W, dtype=np.float32)
    a_src = np.asarray(a_src, dtype=np.float32)
    a_dst = np.asarray(a_dst, dtype=np.float32)
    adj = np.asarray(adj)

    mask = np.where((adj > 0) | np.eye(N, dtype=bool), 0.0, -1e9).astype(np.float32)  # [i, j]
    maskt = np.ascontiguousarray(mask.T.reshape(NCH, 128, N)).astype(BF)  # [jc, jl, i]
    wsd = np.zeros((D, 34), np.float32)
    wsd[:, 0] = W @ a_src
    wsd[:, 33] = W @ a_dst
    identb = np.eye(128, dtype=np.float32)
    onescol = np.ones((128, 1), np.float32)
    cb = np.concatenate([W, wsd, identb, onescol], axis=1).astype(BF)
    # ev_rhs = ev_ps[0:2]*sel[0:2,0]+sel[0:2,1] = [es; ones]
    # ev_lhs = ev_ps[32:34]*sel[32:34,0]+sel[32:34,1] = [ones; ed]
    sel = np.zeros((34, 2), np.float32)
    sel[0] = [1.0, 0.0]
    sel[1] = [0.0, 1.0]
    sel[32] = [0.0, 1.0]
    sel[33] = [1.0, 0.0]
    x16 = x.astype(BF)

    nc = _build_program()

    in_maps = []
    for c in range(NCORES):
        in_maps.append({
            "x": np.ascontiguousarray(x16[c * B_PER_CORE:(c + 1) * B_PER_CORE]),
            "cb": cb, "maskt": maskt, "sel": sel,
        })

    res = run_bass_kernel_spmd(nc, in_maps, list(range(NCORES)))
    out = np.concatenate([res.results[c]["out"] for c in range(NCORES)], axis=0)
    return out  # [B, N, T, F]
